# revision 10
# baseline (speedup 1.0000x reference)
"""Multi-head attention (per-head q/k projections, shared v, head-mean, out-proj)
on Trainium2, data-parallel over batch: 8 batches -> 8 NeuronCores.

Per-core computation (batch b):
  v   = x_value @ Wv.T + bv                          [S, DK]
  qT_h = (x_query @ Wq[h].T + bq[h]).T               [DK, S]
  kT_h = (x_key   @ Wk[h].T + bk[h]).T               [DK, S]
  s_h = (q_h k_h^T)/sqrt(DK), masked, softmax over keys
  attn tensor [S, H, S] (fp32), heads = attn @ v, out = mean_h(heads) @ Wo.T

Design notes:
- Both score orientations come from the same qT/kT tiles by swapping the
  stationary operand: o1 s[sq,sk] (softmax + attn output), o2 st[sk,sq]
  (exp -> Pt bf16 -> AV with v stationary, N=512).
- Masking: additive -1e30 penalties; o1 via DVE tensor_tensor add (PSUM
  evict fused), o2 via an extra identity-stationary matmul accumulating
  penT into PSUM (I.T @ penT = penT).
- Row sums ride the o1 exp (ACT accum_out); 1/sum normalizes the attn
  output on GPSIMD (tensor_scalar, per-partition scalar).
- AV normalization: rec row is transposed (PE), gathered to one partition
  (8 tiny DMAs), partition-broadcast (GPSIMD), then applied with DVE
  tensor_tensor ops while accumulating hbar^T across heads.
- 1/H head-mean folded into host-scaled WoT.
- Matmuls run float32r (1 cyc/row at N>=256); AV and mask-penalty matmuls
  run bf16.
"""

import numpy as np

B, S, D_MODEL, N_HEAD, D_K = 8, 1024, 1024, 8, 128
N_CORES = 8
P = 128
NF = 512  # matmul moving free dim (one PSUM bank of fp32)

SCORE_DTYPE = "f32r"  # "f32r" (fast, ~TF32 rounding) or "f32" (exact, 4x slower)
NEG_BIG = -1.0e30


def mha_tile_kernel(ctx, tc, ins, outs, dims):
    import concourse.mybir as mybir

    nc = tc.nc
    S_, D, H, DK = dims["S"], dims["D"], dims["H"], dims["DK"]
    nf = min(NF, S_, D)
    MC = D // P    # contraction (d_model) chunks
    SC = S_ // P   # seq chunks of 128
    NH = S_ // nf  # seq slices of nf
    DH = D // nf   # d_model slices of nf
    f32 = mybir.dt.float32
    f32r = mybir.dt.float32r
    bf16 = mybir.dt.bfloat16
    Exp = mybir.ActivationFunctionType.Exp
    Identity = mybir.ActivationFunctionType.Identity
    mult = mybir.AluOpType.mult
    add = mybir.AluOpType.add
    SCALE = 1.0 / float(np.sqrt(DK))

    if SCORE_DTYPE == "f32r":
        mmdt = f32r
        fb = lambda ap: ap.bitcast(f32r)
    else:
        mmdt = f32
        fb = lambda ap: ap

    # ---------------- persistent tiles ----------------
    const = ctx.enter_context(tc.tile_pool(name="const", bufs=1))
    eyeB = const.tile([P, P], bf16)   # identity: bf16 transposes + o2 mask matmul
    nc.sync.dma_start(eyeB, ins["eyeB"])
    eyeF = const.tile([P, P], f32)    # identity for f32 transposes (rec rows)
    nc.sync.dma_start(eyeF, ins["eyeI"])
    bqT = const.tile([P, H], f32)
    nc.sync.dma_start(bqT, ins["bqT"])
    bkT = const.tile([P, H], f32)
    nc.sync.dma_start(bkT, ins["bkT"])
    bvc = const.tile([P, 1], f32)
    nc.sync.dma_start(bvc, ins["bv_col"])

    big = ctx.enter_context(tc.tile_pool(name="big", bufs=1))
    qT = big.tile([P, H, S_], mmdt)    # per-head q^T [DK, S]
    kT = big.tile([P, H, S_], mmdt)
    vsb = big.tile([P, SC, DK], bf16)  # v [S, DK] (sk on partitions)
    hbarT = big.tile([P, S_], mmdt)    # sum_h heads_h^T[dv, sq] * rec_h[sq]

    # ---------------- phase 1: projections ----------------
    with (
        tc.tile_pool(name="xin", bufs=1) as xpool,
        tc.tile_pool(name="wqk", bufs=3) as wpool,
        tc.tile_pool(name="vtmp", bufs=1) as vtp,
        tc.tile_pool(name="p1ps", bufs=2, space="PSUM") as p1ps,
        tc.tile_pool(name="pvps", bufs=1, space="PSUM") as pvps,
        tc.tile_pool(name="vtps", bufs=2, space="PSUM") as vtps,
    ):
        wv = wpool.tile([P, MC, DK], mmdt, tag="wv")
        nc.sync.dma_start(wv, fb(ins["wvT"]).rearrange("(c p) k -> p c k", p=P))
        xv = xpool.tile([P, MC, S_], mmdt, tag="xv")
        xq = xpool.tile([P, MC, S_], mmdt, tag="xq")
        xk = xpool.tile([P, MC, S_], mmdt, tag="xk")
        for mc in range(MC):
            nc.sync.dma_start(
                xv[:, mc, :],
                fb(ins["xTv"]).rearrange("(c p) s -> p c s", p=P)[:, mc, :],
            )
        for mc in range(MC):
            nc.sync.dma_start(
                xq[:, mc, :],
                fb(ins["xTq"]).rearrange("(c p) s -> p c s", p=P)[:, mc, :],
            )
        for mc in range(MC):
            nc.sync.dma_start(
                xk[:, mc, :],
                fb(ins["xTk"]).rearrange("(c p) s -> p c s", p=P)[:, mc, :],
            )

        # vT[dv, s] = sum_m WvT[m, dv] xTv[m, s]  (+bv per-partition on evict)
        pvT = pvps.tile([P, S_], f32, tag="pvT")
        for mc in range(MC):
            for nh in range(NH):
                nc.tensor.matmul(
                    pvT[:, nh * nf : (nh + 1) * nf],
                    wv[:, mc, :],
                    xv[:, mc, nh * nf : (nh + 1) * nf],
                    start=(mc == 0),
                    stop=(mc == MC - 1),
                )
        vT = vtp.tile([P, S_], bf16)
        nc.scalar.activation(vT, pvT, Identity, bias=bvc[:, 0:1], scale=1.0)
        # transpose vT -> v [s, dv] per 128-chunk (bf16 PE transposes)
        for sc in range(SC):
            ptv = vtps.tile([P, P], bf16, tag="ptv")
            nc.tensor.transpose(ptv, vT[:, sc * P : (sc + 1) * P], eyeB)
            nc.vector.tensor_copy(vsb[:, sc, :], ptv)

        # qT[h] [DK, S], kT[h] [DK, S]
        for h in range(H):
            wq = wpool.tile([P, MC, DK], mmdt, tag="w")
            nc.sync.dma_start(wq, fb(ins["wqT"])[h].rearrange("(c p) k -> p c k", p=P))
            wk = wpool.tile([P, MC, DK], mmdt, tag="w")
            nc.sync.dma_start(wk, fb(ins["wkT"])[h].rearrange("(c p) k -> p c k", p=P))
            pq = p1ps.tile([P, S_], f32, tag="pq")
            pk = p1ps.tile([P, S_], f32, tag="pq")
            for mc in range(MC):
                for nh in range(NH):
                    nc.tensor.matmul(
                        pq[:, nh * nf : (nh + 1) * nf],
                        wq[:, mc, :],
                        xq[:, mc, nh * nf : (nh + 1) * nf],
                        start=(mc == 0),
                        stop=(mc == MC - 1),
                    )
            for mc in range(MC):
                for nh in range(NH):
                    nc.tensor.matmul(
                        pk[:, nh * nf : (nh + 1) * nf],
                        wk[:, mc, :],
                        xk[:, mc, nh * nf : (nh + 1) * nf],
                        start=(mc == 0),
                        stop=(mc == MC - 1),
                    )
            nc.vector.tensor_scalar_add(qT[:, h, :], pq, bqT[:, h : h + 1])
            nc.vector.tensor_scalar_add(kT[:, h, :], pk, bkT[:, h : h + 1])

    # ---------------- phase 2: attention per head ----------------
    with (
        tc.tile_pool(name="maskp", bufs=1) as maskp,
        tc.tile_pool(name="s1ps", bufs=2, space="PSUM") as s1ps,
        tc.tile_pool(name="s2ps", bufs=2, space="PSUM") as s2ps,
        tc.tile_pool(name="avps", bufs=1, space="PSUM") as avps,
        tc.tile_pool(name="rtps", bufs=1, space="PSUM") as rtps,
        tc.tile_pool(name="sb1", bufs=2) as sb1,
        tc.tile_pool(name="ptp", bufs=2) as ptp,
        tc.tile_pool(name="recp", bufs=2) as recp,
    ):
        # additive penalties (-1e30 at masked positions, 0 elsewhere), bf16:
        # maskpen[sq, sk] for o1 (DVE add), maskTF[sk, sq] for o2 (identity MM)
        maskpen = maskp.tile([P, SC, S_], bf16)
        nc.sync.dma_start(maskpen, ins["maskpen"].rearrange("(c p) s -> p c s", p=P))
        maskTF = maskp.tile([P, SC, S_], bf16)
        nc.sync.dma_start(maskTF, ins["maskT"].rearrange("(c p) s -> p c s", p=P))
        for h in range(H):
            rec = recp.tile([P, SC], f32, tag="rec")
            # --- o1: s[sq, sk] -> +pen (DVE) -> exp+rowsum (ACT) -> norm (GPSIMD)
            for sc in range(SC):
                sm = sb1.tile([P, S_], f32, tag="sm")
                for nh in range(NH):
                    ps1 = s1ps.tile([P, nf], f32, tag="s1")
                    nc.tensor.matmul(
                        ps1,
                        qT[:, h, sc * P : (sc + 1) * P],
                        kT[:, h, nh * nf : (nh + 1) * nf],
                        start=True,
                        stop=True,
                    )
                    nc.vector.tensor_tensor(
                        out=sm[:, nh * nf : (nh + 1) * nf],
                        in0=ps1,
                        in1=maskpen[:, sc, nh * nf : (nh + 1) * nf],
                        op=add,
                    )
                pm = sb1.tile([P, S_], f32, tag="pm")
                sums = sb1.tile([P, 1], f32, tag="sums")
                nc.scalar.activation(pm, sm, Exp, scale=SCALE, accum_out=sums)
                nc.vector.reciprocal(rec[:, sc : sc + 1], sums)
                attn_t = sb1.tile([P, S_], f32, tag="attn")
                nc.gpsimd.tensor_scalar_mul(attn_t, pm, rec[:, sc : sc + 1])
                nc.sync.dma_start(
                    outs["attn"][sc * P : (sc + 1) * P, h, :], attn_t
                )

            # --- rec row: [128, SC] -> transpose -> gather -> broadcast
            prt = rtps.tile([SC, P], f32, tag="rt")
            nc.tensor.transpose(prt, rec, eyeF)
            recT8 = recp.tile([SC, P], f32, tag="recT8")
            nc.vector.tensor_copy(recT8, prt)
            recrow = recp.tile([1, S_], f32, tag="recrow")
            for c in range(SC):
                nc.sync.dma_start(
                    recrow[0:1, c * P : (c + 1) * P], recT8[c : c + 1, :]
                )
            rb = recp.tile([P, S_], f32, tag="rb")
            nc.gpsimd.partition_broadcast(rb, recrow)

            # --- o2: st[sk, sq] + penT (identity MM) -> exp -> Pt bf16
            pt = ptp.tile([P, SC, S_], bf16, tag="pt")
            for kc in range(SC):
                ps2 = s2ps.tile([P, S_], f32, tag="s2")
                for nh in range(NH):
                    nc.tensor.matmul(
                        ps2[:, nh * nf : (nh + 1) * nf],
                        kT[:, h, kc * P : (kc + 1) * P],
                        qT[:, h, nh * nf : (nh + 1) * nf],
                        start=True,
                        stop=False,
                    )
                    nc.tensor.matmul(
                        ps2[:, nh * nf : (nh + 1) * nf],
                        eyeB,
                        maskTF[:, kc, nh * nf : (nh + 1) * nf],
                        start=False,
                        stop=True,
                    )
                nc.scalar.activation(pt[:, kc, :], ps2, Exp, scale=SCALE)

            # --- AV: headsU^T[dv, sq] = sum_sk v[sk, dv] Pt[sk, sq]; then
            #     hbarT += headsU^T * rb  (per-head normalization)
            for nh in range(NH):
                pav = avps.tile([P, nf], f32, tag="av")
                for kc in range(SC):
                    nc.tensor.matmul(
                        pav,
                        vsb[:, kc, :],
                        pt[:, kc, nh * nf : (nh + 1) * nf],
                        start=(kc == 0),
                        stop=(kc == SC - 1),
                    )
                sl = slice(nh * nf, (nh + 1) * nf)
                if h == 0:
                    nc.vector.tensor_tensor(
                        out=hbarT[:, sl], in0=pav, in1=rb[:, sl], op=mult
                    )
                else:
                    tmp = sb1.tile([P, nf], f32, tag="tmp")
                    nc.vector.tensor_tensor(
                        out=tmp, in0=pav, in1=rb[:, sl], op=mult
                    )
                    nc.vector.tensor_tensor(
                        out=hbarT[:, sl], in0=hbarT[:, sl], in1=tmp, op=add
                    )

    # ---------------- phase 3: out projection ----------------
    with (
        tc.tile_pool(name="ops", bufs=2, space="PSUM") as ops,
        tc.tile_pool(name="wop", bufs=1) as wop,
        tc.tile_pool(name="osb", bufs=3) as osbp,
    ):
        woT = wop.tile([P, D], mmdt)
        nc.sync.dma_start(woT, fb(ins["woT"]))
        for sc in range(SC):
            for dh in range(DH):
                po = ops.tile([P, nf], f32, tag="po")
                nc.tensor.matmul(
                    po,
                    hbarT[:, sc * P : (sc + 1) * P],
                    woT[:, dh * nf : (dh + 1) * nf],
                    start=True,
                    stop=True,
                )
                ot = osbp.tile([P, nf], f32, tag="ot")
                nc.scalar.copy(ot, po)
                nc.sync.dma_start(
                    outs["out"][sc * P : (sc + 1) * P, dh * nf : (dh + 1) * nf], ot
                )


def _declare_tensors(nc, dims):
    import concourse.mybir as mybir

    S_, D, H, DK = dims["S"], dims["D"], dims["H"], dims["DK"]
    f32 = mybir.dt.float32
    bf16 = mybir.dt.bfloat16
    ins = {}
    for name, shape, dt in [
        ("xTq", [D, S_], f32),
        ("xTk", [D, S_], f32),
        ("xTv", [D, S_], f32),
        ("wqT", [H, D, DK], f32),
        ("wkT", [H, D, DK], f32),
        ("wvT", [D, DK], f32),
        ("woT", [DK, D], f32),
        ("bqT", [DK, H], f32),
        ("bkT", [DK, H], f32),
        ("bv_col", [DK, 1], f32),
        ("eyeI", [P, P], f32),
        ("eyeB", [P, P], bf16),
        ("maskpen", [S_, S_], bf16),
        ("maskT", [S_, S_], bf16),
    ]:
        ins[name] = nc.dram_tensor(name, shape, dt, kind="ExternalInput").ap()
    outs = {
        "out": nc.dram_tensor("out", [S_, D], f32, kind="ExternalOutput").ap(),
        "attn": nc.dram_tensor("attn", [S_, H, S_], f32, kind="ExternalOutput").ap(),
    }
    return ins, outs


def build_bass(dims=None):
    from contextlib import ExitStack

    import concourse.bacc as bacc
    import concourse.tile as tile

    if dims is None:
        dims = {"S": S, "D": D_MODEL, "H": N_HEAD, "DK": D_K}
    nc = bacc.Bacc("TRN2", target_bir_lowering=False)
    ins, outs = _declare_tensors(nc, dims)
    with tile.TileContext(nc) as tc:
        with ExitStack() as ctx:
            mha_tile_kernel(ctx, tc, ins, outs, dims)
    nc.compile()
    return nc


def host_prep(x_query, x_key, x_value, mask, Wq, bq, Wk, bk, Wv, bv, Wo):
    """Build the per-core input maps (host-side layout prep, numpy only)."""
    import ml_dtypes

    f = np.float32
    bf = ml_dtypes.bfloat16
    x_query = np.asarray(x_query, f)
    x_key = np.asarray(x_key, f)
    x_value = np.asarray(x_value, f)
    mask_b = np.asarray(mask, bool)
    H = np.asarray(Wq).shape[0]
    xTq = np.ascontiguousarray(x_query.transpose(0, 2, 1))
    xTk = np.ascontiguousarray(x_key.transpose(0, 2, 1))
    xTv = np.ascontiguousarray(x_value.transpose(0, 2, 1))
    pen = np.float32(NEG_BIG)
    maskpen = (mask_b.astype(f) * pen).astype(bf)
    maskT = (
        np.ascontiguousarray(mask_b.transpose(0, 2, 1)).astype(f) * pen
    ).astype(bf)
    wqT = np.ascontiguousarray(np.asarray(Wq, f).transpose(0, 2, 1))
    wkT = np.ascontiguousarray(np.asarray(Wk, f).transpose(0, 2, 1))
    wvT = np.ascontiguousarray(np.asarray(Wv, f).T)
    woT = np.ascontiguousarray(np.asarray(Wo, f).T) * np.float32(1.0 / H)
    bqT = np.ascontiguousarray(np.asarray(bq, f).T)
    bkT = np.ascontiguousarray(np.asarray(bk, f).T)
    bv_col = np.asarray(bv, f).reshape(-1, 1)
    eyeI = np.eye(P, dtype=f)
    eyeB = np.eye(P, dtype=f).astype(bf)

    nb = x_query.shape[0]
    in_maps = []
    for b in range(nb):
        in_maps.append(
            {
                "xTq": xTq[b],
                "xTk": xTk[b],
                "xTv": xTv[b],
                "wqT": wqT,
                "wkT": wkT,
                "wvT": wvT,
                "woT": woT,
                "bqT": bqT,
                "bkT": bkT,
                "bv_col": bv_col,
                "eyeI": eyeI,
                "eyeB": eyeB,
                "maskpen": maskpen[b],
                "maskT": maskT[b],
            }
        )
    return in_maps


_CACHED = {}


def kernel(x_query, x_key, x_value, mask, Wq, bq, Wk, bk, Wv, bv, Wo, trace=False):
    from concourse.bass_utils import run_bass_kernel_spmd

    if "nc" not in _CACHED:
        _CACHED["nc"] = build_bass()
    nc = _CACHED["nc"]
    in_maps = host_prep(
        x_query, x_key, x_value, mask, Wq, bq, Wk, bk, Wv, bv, Wo
    )
    res = run_bass_kernel_spmd(
        nc, in_maps, core_ids=list(range(N_CORES)), trace=trace
    )
    outputs = np.stack([r["out"] for r in res.results])
    attns = np.stack([r["attn"] for r in res.results])
    if trace:
        kernel.last_results = res
    return outputs, attns


# revision 11
# speedup vs baseline: 3.3536x; 3.3536x over previous
"""Multi-head attention (per-head q/k projections, shared v, head-mean, out-proj)
on Trainium2, data-parallel over batch: 8 batches -> 8 NeuronCores.

Per-core computation (batch b):
  v   = x_value @ Wv.T + bv                          [S, DK]
  qT_h = (x_query @ Wq[h].T + bq[h]).T               [DK, S]
  kT_h = (x_key   @ Wk[h].T + bk[h]).T               [DK, S]
  s_h = (q_h k_h^T)/sqrt(DK), masked, softmax over keys
  attn tensor [S, H, S] (fp32), heads = attn @ v, out = mean_h(heads) @ Wo.T

Design notes:
- Both score orientations come from the same qT/kT tiles by swapping the
  stationary operand: o1 s[sq,sk] (softmax + attn output), o2 st[sk,sq]
  (exp -> Pt bf16 -> AV with v stationary, N=512).
- Masking: additive -1e30 penalties; o1 via DVE tensor_tensor add (PSUM
  evict fused), o2 via an extra identity-stationary matmul accumulating
  penT into PSUM (I.T @ penT = penT).
- Row sums ride the o1 exp (ACT accum_out); 1/sum normalizes the attn
  output on GPSIMD (tensor_scalar, per-partition scalar).
- AV normalization: rec row is transposed (PE), gathered to one partition
  (8 tiny DMAs), partition-broadcast (GPSIMD), then applied with DVE
  tensor_tensor ops while accumulating hbar^T across heads.
- 1/H head-mean folded into host-scaled WoT.
- Matmuls run float32r (1 cyc/row at N>=256); AV and mask-penalty matmuls
  run bf16.
"""

import numpy as np

B, S, D_MODEL, N_HEAD, D_K = 8, 1024, 1024, 8, 128
N_CORES = 8
P = 128
NF = 512  # matmul moving free dim (one PSUM bank of fp32)

SCORE_DTYPE = "f32r"  # "f32r" (fast, ~TF32 rounding) or "f32" (exact, 4x slower)
NEG_BIG = -1.0e30


def mha_tile_kernel(ctx, tc, ins, outs, dims):
    import concourse.mybir as mybir

    nc = tc.nc
    S_, D, H, DK = dims["S"], dims["D"], dims["H"], dims["DK"]
    nf = min(NF, S_, D)
    MC = D // P    # contraction (d_model) chunks
    SC = S_ // P   # seq chunks of 128
    NH = S_ // nf  # seq slices of nf
    DH = D // nf   # d_model slices of nf
    f32 = mybir.dt.float32
    f32r = mybir.dt.float32r
    bf16 = mybir.dt.bfloat16
    Exp = mybir.ActivationFunctionType.Exp
    Identity = mybir.ActivationFunctionType.Identity
    mult = mybir.AluOpType.mult
    add = mybir.AluOpType.add
    SCALE = 1.0 / float(np.sqrt(DK))

    if SCORE_DTYPE == "f32r":
        mmdt = f32r
        fb = lambda ap: ap.bitcast(f32r)
    else:
        mmdt = f32
        fb = lambda ap: ap

    # ---------------- persistent tiles ----------------
    const = ctx.enter_context(tc.tile_pool(name="const", bufs=1))
    eyeB = const.tile([P, P], bf16)   # identity: bf16 transposes + o2 mask matmul
    nc.sync.dma_start(eyeB, ins["eyeB"])
    eyeF = const.tile([P, P], f32)    # identity for f32 transposes (rec rows)
    nc.sync.dma_start(eyeF, ins["eyeI"])
    bqT = const.tile([P, H], f32)
    nc.sync.dma_start(bqT, ins["bqT"])
    bkT = const.tile([P, H], f32)
    nc.sync.dma_start(bkT, ins["bkT"])
    bvc = const.tile([P, 1], f32)
    nc.sync.dma_start(bvc, ins["bv_col"])

    big = ctx.enter_context(tc.tile_pool(name="big", bufs=1))
    qT = big.tile([P, H, S_], mmdt)    # per-head q^T [DK, S]
    kT = big.tile([P, H, S_], mmdt)
    vsb = big.tile([P, SC, DK], bf16)  # v [S, DK] (sk on partitions)
    hbarT = big.tile([P, S_], mmdt)    # sum_h heads_h^T[dv, sq] * rec_h[sq]

    # ---------------- phase 1: projections ----------------
    with (
        tc.tile_pool(name="xin", bufs=1) as xpool,
        tc.tile_pool(name="wqk", bufs=3) as wpool,
        tc.tile_pool(name="vtmp", bufs=1) as vtp,
        tc.tile_pool(name="p1ps", bufs=2, space="PSUM") as p1ps,
        tc.tile_pool(name="pvps", bufs=1, space="PSUM") as pvps,
        tc.tile_pool(name="vtps", bufs=2, space="PSUM") as vtps,
    ):
        wv = wpool.tile([P, MC, DK], mmdt, tag="wv")
        nc.sync.dma_start(wv, fb(ins["wvT"]).rearrange("(c p) k -> p c k", p=P))
        xv = xpool.tile([P, MC, S_], mmdt, tag="xv")
        xq = xpool.tile([P, MC, S_], mmdt, tag="xq")
        xk = xpool.tile([P, MC, S_], mmdt, tag="xk")
        for mc in range(MC):
            nc.sync.dma_start(
                xv[:, mc, :],
                fb(ins["xTv"]).rearrange("(c p) s -> p c s", p=P)[:, mc, :],
            )
        for mc in range(MC):
            nc.sync.dma_start(
                xq[:, mc, :],
                fb(ins["xTq"]).rearrange("(c p) s -> p c s", p=P)[:, mc, :],
            )
        for mc in range(MC):
            nc.sync.dma_start(
                xk[:, mc, :],
                fb(ins["xTk"]).rearrange("(c p) s -> p c s", p=P)[:, mc, :],
            )

        # vT[dv, s] = sum_m WvT[m, dv] xTv[m, s]  (+bv per-partition on evict)
        pvT = pvps.tile([P, S_], f32, tag="pvT")
        for mc in range(MC):
            for nh in range(NH):
                nc.tensor.matmul(
                    pvT[:, nh * nf : (nh + 1) * nf],
                    wv[:, mc, :],
                    xv[:, mc, nh * nf : (nh + 1) * nf],
                    start=(mc == 0),
                    stop=(mc == MC - 1),
                )
        vT = vtp.tile([P, S_], bf16)
        nc.scalar.activation(vT, pvT, Identity, bias=bvc[:, 0:1], scale=1.0)
        # transpose vT -> v [s, dv] per 128-chunk (bf16 PE transposes)
        for sc in range(SC):
            ptv = vtps.tile([P, P], bf16, tag="ptv")
            nc.tensor.transpose(ptv, vT[:, sc * P : (sc + 1) * P], eyeB)
            nc.vector.tensor_copy(vsb[:, sc, :], ptv)

        # qT[h] [DK, S], kT[h] [DK, S]
        for h in range(H):
            wq = wpool.tile([P, MC, DK], mmdt, tag="w")
            nc.sync.dma_start(wq, fb(ins["wqT"])[h].rearrange("(c p) k -> p c k", p=P))
            wk = wpool.tile([P, MC, DK], mmdt, tag="w")
            nc.sync.dma_start(wk, fb(ins["wkT"])[h].rearrange("(c p) k -> p c k", p=P))
            pq = p1ps.tile([P, S_], f32, tag="pq")
            pk = p1ps.tile([P, S_], f32, tag="pq")
            for mc in range(MC):
                for nh in range(NH):
                    nc.tensor.matmul(
                        pq[:, nh * nf : (nh + 1) * nf],
                        wq[:, mc, :],
                        xq[:, mc, nh * nf : (nh + 1) * nf],
                        start=(mc == 0),
                        stop=(mc == MC - 1),
                    )
            for mc in range(MC):
                for nh in range(NH):
                    nc.tensor.matmul(
                        pk[:, nh * nf : (nh + 1) * nf],
                        wk[:, mc, :],
                        xk[:, mc, nh * nf : (nh + 1) * nf],
                        start=(mc == 0),
                        stop=(mc == MC - 1),
                    )
            nc.vector.tensor_scalar_add(qT[:, h, :], pq, bqT[:, h : h + 1])
            nc.vector.tensor_scalar_add(kT[:, h, :], pk, bkT[:, h : h + 1])

    # ---------------- phase 2: attention per head ----------------
    with (
        tc.tile_pool(name="maskp", bufs=1) as maskp,
        tc.tile_pool(name="s1ps", bufs=2, space="PSUM") as s1ps,
        tc.tile_pool(name="s2ps", bufs=2, space="PSUM") as s2ps,
        tc.tile_pool(name="avps", bufs=1, space="PSUM") as avps,
        tc.tile_pool(name="rtps", bufs=1, space="PSUM") as rtps,
        tc.tile_pool(name="sb1", bufs=2) as sb1,
        tc.tile_pool(name="ptp", bufs=2) as ptp,
        tc.tile_pool(name="recp", bufs=2) as recp,
    ):
        # additive penalties (-1e30 at masked positions, 0 elsewhere), bf16:
        # maskpen[sq, sk] for o1 (DVE add), maskTF[sk, sq] for o2 (identity MM)
        maskpen = maskp.tile([P, SC, S_], f32)
        nc.sync.dma_start(maskpen, ins["maskpen"].rearrange("(c p) s -> p c s", p=P))
        maskTF = maskp.tile([P, SC, S_], bf16)
        nc.sync.dma_start(maskTF, ins["maskT"].rearrange("(c p) s -> p c s", p=P))
        for h in range(H):
            rec = recp.tile([P, SC], f32, tag="rec")
            # --- o1: s[sq, sk] -> +pen (DVE) -> exp+rowsum (ACT) -> norm (GPSIMD)
            for sc in range(SC):
                sm = sb1.tile([P, S_], f32, tag="sm")
                for nh in range(NH):
                    ps1 = s1ps.tile([P, nf], f32, tag="s1")
                    nc.tensor.matmul(
                        ps1,
                        qT[:, h, sc * P : (sc + 1) * P],
                        kT[:, h, nh * nf : (nh + 1) * nf],
                        start=True,
                        stop=True,
                    )
                    nc.vector.tensor_tensor(
                        out=sm[:, nh * nf : (nh + 1) * nf],
                        in0=ps1,
                        in1=maskpen[:, sc, nh * nf : (nh + 1) * nf],
                        op=add,
                    )
                pm = sb1.tile([P, S_], f32, tag="pm")
                sums = sb1.tile([P, 1], f32, tag="sums")
                nc.scalar.activation(pm, sm, Exp, scale=SCALE, accum_out=sums)
                nc.vector.reciprocal(rec[:, sc : sc + 1], sums)
                attn_t = sb1.tile([P, S_], f32, tag="attn")
                nc.vector.tensor_scalar_mul(attn_t, pm, rec[:, sc : sc + 1])
                nc.sync.dma_start(
                    outs["attn"][sc * P : (sc + 1) * P, h, :], attn_t
                )

            # --- rec row: [128, SC] -> transpose -> gather -> broadcast
            prt = rtps.tile([SC, P], f32, tag="rt")
            nc.tensor.transpose(prt, rec, eyeF)
            recT8 = recp.tile([SC, P], f32, tag="recT8")
            nc.vector.tensor_copy(recT8, prt)
            recrow = recp.tile([1, S_], f32, tag="recrow")
            for c in range(SC):
                nc.sync.dma_start(
                    recrow[0:1, c * P : (c + 1) * P], recT8[c : c + 1, :]
                )
            rb = recp.tile([P, S_], f32, tag="rb")
            nc.gpsimd.partition_broadcast(rb, recrow)

            # --- o2: st[sk, sq] + penT (identity MM) -> exp -> Pt bf16
            pt = ptp.tile([P, SC, S_], bf16, tag="pt")
            for kc in range(SC):
                ps2 = s2ps.tile([P, S_], f32, tag="s2")
                for nh in range(NH):
                    nc.tensor.matmul(
                        ps2[:, nh * nf : (nh + 1) * nf],
                        kT[:, h, kc * P : (kc + 1) * P],
                        qT[:, h, nh * nf : (nh + 1) * nf],
                        start=True,
                        stop=False,
                    )
                    nc.tensor.matmul(
                        ps2[:, nh * nf : (nh + 1) * nf],
                        eyeB,
                        maskTF[:, kc, nh * nf : (nh + 1) * nf],
                        start=False,
                        stop=True,
                    )
                nc.scalar.activation(pt[:, kc, :], ps2, Exp, scale=SCALE)

            # --- AV: headsU^T[dv, sq] = sum_sk v[sk, dv] Pt[sk, sq]; then
            #     hbarT += headsU^T * rb  (per-head normalization)
            for nh in range(NH):
                pav = avps.tile([P, nf], f32, tag="av")
                for kc in range(SC):
                    nc.tensor.matmul(
                        pav,
                        vsb[:, kc, :],
                        pt[:, kc, nh * nf : (nh + 1) * nf],
                        start=(kc == 0),
                        stop=(kc == SC - 1),
                    )
                sl = slice(nh * nf, (nh + 1) * nf)
                if h == 0:
                    nc.vector.tensor_tensor(
                        out=hbarT[:, sl], in0=pav, in1=rb[:, sl], op=mult
                    )
                else:
                    tmp = sb1.tile([P, nf], f32, tag="tmp")
                    nc.vector.tensor_tensor(
                        out=tmp, in0=pav, in1=rb[:, sl], op=mult
                    )
                    nc.vector.tensor_tensor(
                        out=hbarT[:, sl], in0=hbarT[:, sl], in1=tmp, op=add
                    )

    # ---------------- phase 3: out projection ----------------
    with (
        tc.tile_pool(name="ops", bufs=2, space="PSUM") as ops,
        tc.tile_pool(name="wop", bufs=1) as wop,
        tc.tile_pool(name="osb", bufs=3) as osbp,
    ):
        woT = wop.tile([P, D], mmdt)
        nc.sync.dma_start(woT, fb(ins["woT"]))
        for sc in range(SC):
            for dh in range(DH):
                po = ops.tile([P, nf], f32, tag="po")
                nc.tensor.matmul(
                    po,
                    hbarT[:, sc * P : (sc + 1) * P],
                    woT[:, dh * nf : (dh + 1) * nf],
                    start=True,
                    stop=True,
                )
                ot = osbp.tile([P, nf], f32, tag="ot")
                nc.scalar.copy(ot, po)
                nc.sync.dma_start(
                    outs["out"][sc * P : (sc + 1) * P, dh * nf : (dh + 1) * nf], ot
                )


def _declare_tensors(nc, dims):
    import concourse.mybir as mybir

    S_, D, H, DK = dims["S"], dims["D"], dims["H"], dims["DK"]
    f32 = mybir.dt.float32
    bf16 = mybir.dt.bfloat16
    ins = {}
    for name, shape, dt in [
        ("xTq", [D, S_], f32),
        ("xTk", [D, S_], f32),
        ("xTv", [D, S_], f32),
        ("wqT", [H, D, DK], f32),
        ("wkT", [H, D, DK], f32),
        ("wvT", [D, DK], f32),
        ("woT", [DK, D], f32),
        ("bqT", [DK, H], f32),
        ("bkT", [DK, H], f32),
        ("bv_col", [DK, 1], f32),
        ("eyeI", [P, P], f32),
        ("eyeB", [P, P], bf16),
        ("maskpen", [S_, S_], f32),
        ("maskT", [S_, S_], bf16),
    ]:
        ins[name] = nc.dram_tensor(name, shape, dt, kind="ExternalInput").ap()
    outs = {
        "out": nc.dram_tensor("out", [S_, D], f32, kind="ExternalOutput").ap(),
        "attn": nc.dram_tensor("attn", [S_, H, S_], f32, kind="ExternalOutput").ap(),
    }
    return ins, outs


def build_bass(dims=None):
    from contextlib import ExitStack

    import concourse.bacc as bacc
    import concourse.tile as tile

    if dims is None:
        dims = {"S": S, "D": D_MODEL, "H": N_HEAD, "DK": D_K}
    nc = bacc.Bacc("TRN2", target_bir_lowering=False)
    ins, outs = _declare_tensors(nc, dims)
    with tile.TileContext(nc) as tc:
        with ExitStack() as ctx:
            mha_tile_kernel(ctx, tc, ins, outs, dims)
    nc.compile()
    return nc


def host_prep(x_query, x_key, x_value, mask, Wq, bq, Wk, bk, Wv, bv, Wo):
    """Build the per-core input maps (host-side layout prep, numpy only)."""
    import ml_dtypes

    f = np.float32
    bf = ml_dtypes.bfloat16
    x_query = np.asarray(x_query, f)
    x_key = np.asarray(x_key, f)
    x_value = np.asarray(x_value, f)
    mask_b = np.asarray(mask, bool)
    H = np.asarray(Wq).shape[0]
    xTq = np.ascontiguousarray(x_query.transpose(0, 2, 1))
    xTk = np.ascontiguousarray(x_key.transpose(0, 2, 1))
    xTv = np.ascontiguousarray(x_value.transpose(0, 2, 1))
    pen = np.float32(NEG_BIG)
    maskpen = mask_b.astype(f) * pen
    maskT = (
        np.ascontiguousarray(mask_b.transpose(0, 2, 1)).astype(f) * pen
    ).astype(bf)
    wqT = np.ascontiguousarray(np.asarray(Wq, f).transpose(0, 2, 1))
    wkT = np.ascontiguousarray(np.asarray(Wk, f).transpose(0, 2, 1))
    wvT = np.ascontiguousarray(np.asarray(Wv, f).T)
    woT = np.ascontiguousarray(np.asarray(Wo, f).T) * np.float32(1.0 / H)
    bqT = np.ascontiguousarray(np.asarray(bq, f).T)
    bkT = np.ascontiguousarray(np.asarray(bk, f).T)
    bv_col = np.asarray(bv, f).reshape(-1, 1)
    eyeI = np.eye(P, dtype=f)
    eyeB = np.eye(P, dtype=f).astype(bf)

    nb = x_query.shape[0]
    in_maps = []
    for b in range(nb):
        in_maps.append(
            {
                "xTq": xTq[b],
                "xTk": xTk[b],
                "xTv": xTv[b],
                "wqT": wqT,
                "wkT": wkT,
                "wvT": wvT,
                "woT": woT,
                "bqT": bqT,
                "bkT": bkT,
                "bv_col": bv_col,
                "eyeI": eyeI,
                "eyeB": eyeB,
                "maskpen": maskpen[b],
                "maskT": maskT[b],
            }
        )
    return in_maps


_CACHED = {}


def kernel(x_query, x_key, x_value, mask, Wq, bq, Wk, bk, Wv, bv, Wo, trace=False):
    from concourse.bass_utils import run_bass_kernel_spmd

    if "nc" not in _CACHED:
        _CACHED["nc"] = build_bass()
    nc = _CACHED["nc"]
    in_maps = host_prep(
        x_query, x_key, x_value, mask, Wq, bq, Wk, bk, Wv, bv, Wo
    )
    res = run_bass_kernel_spmd(
        nc, in_maps, core_ids=list(range(N_CORES)), trace=trace
    )
    outputs = np.stack([r["out"] for r in res.results])
    attns = np.stack([r["attn"] for r in res.results])
    if trace:
        kernel.last_results = res
    return outputs, attns
